# revision 24
# baseline (speedup 1.0000x reference)
"""Trainium2 Bass kernel for nn_AbstractRelu (DeepPoly abstract-ReLU transform).

The reference's piecewise-linear transform reduces exactly to:
    x_out    = relu(x)
    high_out = relu(high)        (crossing branch: w_high*high + b_high == high)
    low_out  = low if low + high >= 0 else 0
and `relu(high)` can replace `high` in the low_out test without changing any
result (when high <= 0, low < high <= 0 forces low + high < 0 AND low < 0).

Sharding: N=16.7M elements split evenly across 8 NeuronCores; fully
elementwise, no communication.
"""

import numpy as np

import concourse.bass as bass
import concourse.bacc as bacc
import concourse.mybir as mybir
from concourse.tile import TileContext
from concourse.bass_utils import run_bass_kernel_spmd

N = 16777216
N_CORES = 8
SHARD = N // N_CORES  # 2_097_152
P = 128
FREE = SHARD // P  # 16384 f32 per partition per core (64 KiB)
TILE_COLS = 4096  # 2 MiB per DMA transfer
F32 = mybir.dt.float32


def build_program(
    free: int = FREE,
    tile_cols: int = TILE_COLS,
    bufs: int = 3,
    repeats: int = 1,
    hw_loop_repeats: int = 1,
    inplace_low: bool = False,
    store_engine: str = "gpsimd",
    load_engine: str = "split",
    dma_map: str | None = None,
    x_relu_on_dve: bool = False,
    tail_split: int = 1,
) -> bass.Bass:
    """hw_loop_repeats wraps the whole body in a tc.For_i hardware loop —
    used only by the timing harness (repeat-differencing).
    inplace_low computes low_out inside the high tile (3 SBUF tags instead
    of 4, allowing larger tiles)."""
    assert free % tile_cols == 0
    n_tiles = free // tile_cols

    nc = bacc.Bacc(
        "TRN2", target_bir_lowering=False, debug=False, num_devices=N_CORES
    )
    x = nc.declare_dram_parameter("x", [P, free], F32, isOutput=False)
    low = nc.declare_dram_parameter("low", [P, free], F32, isOutput=False)
    high = nc.declare_dram_parameter("high", [P, free], F32, isOutput=False)
    x_out = nc.declare_dram_parameter("x_out", [P, free], F32, isOutput=True)
    low_out = nc.declare_dram_parameter("low_out", [P, free], F32, isOutput=True)
    high_out = nc.declare_dram_parameter("high_out", [P, free], F32, isOutput=True)

    relu = mybir.ActivationFunctionType.Relu
    with TileContext(nc) as tc:
        with tc.tile_pool(name="io", bufs=bufs) as pool:
            engines = {"scalar": nc.scalar, "gpsimd": nc.gpsimd, "sync": nc.sync}

            def eng_for(stream: str, t: int):
                """Resolve the DMA-issuing engine for stream in
                {x,h,l,xo,ho,lo} at tile t."""
                if dma_map is not None:
                    spec = dict(kv.split(":") for kv in dma_map.split(","))
                    e = spec[stream]
                    if e == "alt":  # alternate HWDGE rings by parity
                        e = "sync" if t % 2 == 0 else "scalar"
                    return engines[e]
                if stream in ("x", "h", "l"):
                    if load_engine == "split":
                        return engines["scalar" if stream == "x" else "sync"]
                    return engines[load_engine]
                if store_engine == "mix":
                    return engines["scalar" if stream == "xo" else "gpsimd"]
                if store_engine == "alt":
                    return engines["gpsimd" if t % 2 == 0 else "scalar"]
                return engines[store_engine]

            # (offset, width) chunks; the last tile optionally split so the
            # final load->DVE->store chain is short (pipelines the drain).
            chunks = [(i * tile_cols, tile_cols) for i in range(n_tiles - 1)]
            if tail_split > 1:
                sub = tile_cols // tail_split
                base = (n_tiles - 1) * tile_cols
                chunks += [(base + j * sub, sub) for j in range(tail_split)]
            else:
                chunks.append(((n_tiles - 1) * tile_cols, tile_cols))

            def body():
                for t in range(len(chunks) * repeats):
                    off, width = chunks[t % len(chunks)]
                    sl = slice(off, off + width)
                    x_store = eng_for("xo", t)
                    store_ho = eng_for("ho", t)
                    store_lo = eng_for("lo", t)

                    xt = pool.tile([P, width], F32, tag="x")
                    eng_for("x", t).dma_start(out=xt[:], in_=x[:, sl])
                    if x_relu_on_dve:
                        nc.vector.tensor_scalar_max(xt[:], xt[:], 0.0)
                    else:
                        nc.scalar.activation(xt[:], xt[:], relu)
                    x_store.dma_start(out=x_out[:, sl], in_=xt[:])

                    ht = pool.tile([P, width], F32, tag="h")
                    eng_for("h", t).dma_start(out=ht[:], in_=high[:, sl])
                    lt = pool.tile([P, width], F32, tag="l")
                    eng_for("l", t).dma_start(out=lt[:], in_=low[:, sl])

                    nc.scalar.activation(ht[:], ht[:], relu)
                    store_ho.dma_start(out=high_out[:, sl], in_=ht[:])

                    tt = ht if inplace_low else pool.tile([P, width], F32, tag="t")
                    nc.vector.tensor_add(tt[:], lt[:], ht[:])
                    nc.vector.tensor_scalar(
                        tt[:], tt[:], 0.0, None, mybir.AluOpType.is_ge
                    )
                    nc.vector.tensor_mul(tt[:], tt[:], lt[:])
                    store_lo.dma_start(out=low_out[:, sl], in_=tt[:])

            if hw_loop_repeats > 1:
                with tc.For_i(0, hw_loop_repeats, 1):
                    body()
            else:
                body()
    nc.compile()
    return nc


_NC = None


def _get_nc() -> bass.Bass:
    global _NC
    if _NC is None:
        _NC = build_program()
    return _NC


def kernel(x: np.ndarray, low: np.ndarray, high: np.ndarray, **_run_kwargs):
    nc = _get_nc()
    x = np.ascontiguousarray(np.asarray(x, dtype=np.float32).reshape(-1))
    low = np.ascontiguousarray(np.asarray(low, dtype=np.float32).reshape(-1))
    high = np.ascontiguousarray(np.asarray(high, dtype=np.float32).reshape(-1))
    assert x.shape == (N,), x.shape
    in_maps = []
    for c in range(N_CORES):
        s = slice(c * SHARD, (c + 1) * SHARD)
        in_maps.append(
            {
                "x": x[s].reshape(P, FREE),
                "low": low[s].reshape(P, FREE),
                "high": high[s].reshape(P, FREE),
            }
        )
    res = run_bass_kernel_spmd(nc, in_maps, list(range(N_CORES)), **_run_kwargs)
    results = res.results
    x_out = np.concatenate([results[c]["x_out"].reshape(-1) for c in range(N_CORES)])
    low_out = np.concatenate([results[c]["low_out"].reshape(-1) for c in range(N_CORES)])
    high_out = np.concatenate([results[c]["high_out"].reshape(-1) for c in range(N_CORES)])
    if _run_kwargs:
        kernel.last_results = res  # expose trace/profile to test harness
    return (
        x_out.astype(np.float32, copy=False),
        low_out.astype(np.float32, copy=False),
        high_out.astype(np.float32, copy=False),
    )


# revision 25
# speedup vs baseline: 1.5303x; 1.5303x over previous
"""Trainium2 Bass kernel for nn_AbstractRelu (DeepPoly abstract-ReLU transform).

The reference's piecewise-linear transform reduces exactly to:
    x_out    = relu(x)
    high_out = relu(high)        (crossing branch: w_high*high + b_high == high)
    low_out  = low if low + high >= 0 else 0
and `relu(high)` can replace `high` in the low_out test without changing any
result (when high <= 0, low < high <= 0 forces low + high < 0 AND low < 0).

Sharding: N=16.7M elements split evenly across 8 NeuronCores; fully
elementwise, no communication.
"""

import numpy as np

import concourse.bass as bass
import concourse.bacc as bacc
import concourse.mybir as mybir
from concourse.tile import TileContext
from concourse.bass_utils import run_bass_kernel_spmd

N = 16777216
N_CORES = 8
SHARD = N // N_CORES  # 2_097_152
P = 128
FREE = SHARD // P  # 16384 f32 per partition per core (64 KiB)
TILE_COLS = 4096  # 2 MiB per DMA transfer
F32 = mybir.dt.float32


def build_program(
    free: int = FREE,
    tile_cols: int = TILE_COLS,
    bufs: int = 3,
    repeats: int = 1,
    hw_loop_repeats: int = 1,
    inplace_low: bool = False,
    store_engine: str = "gpsimd",
    load_engine: str = "split",
    dma_map: str | None = None,
    x_relu_on_dve: bool = False,
    tail_split: int = 1,
) -> bass.Bass:
    """hw_loop_repeats wraps the whole body in a tc.For_i hardware loop —
    used only by the timing harness (repeat-differencing).
    inplace_low computes low_out inside the high tile (3 SBUF tags instead
    of 4, allowing larger tiles)."""
    assert free % tile_cols == 0
    n_tiles = free // tile_cols

    nc = bacc.Bacc(
        "TRN2", target_bir_lowering=False, debug=False, num_devices=N_CORES
    )
    x = nc.declare_dram_parameter("x", [P, free], F32, isOutput=False)
    low = nc.declare_dram_parameter("low", [P, free], F32, isOutput=False)
    high = nc.declare_dram_parameter("high", [P, free], F32, isOutput=False)
    x_out = nc.declare_dram_parameter("x_out", [P, free], F32, isOutput=True)
    low_out = nc.declare_dram_parameter("low_out", [P, free], F32, isOutput=True)
    high_out = nc.declare_dram_parameter("high_out", [P, free], F32, isOutput=True)

    relu = mybir.ActivationFunctionType.Relu
    with TileContext(nc) as tc:
        with tc.tile_pool(name="io", bufs=bufs) as pool:
            engines = {"scalar": nc.scalar, "gpsimd": nc.gpsimd, "sync": nc.sync}

            def eng_for(stream: str, t: int):
                """Resolve the DMA-issuing engine for stream in
                {x,h,l,xo,ho,lo} at tile t."""
                if dma_map is not None:
                    spec = dict(kv.split(":") for kv in dma_map.split(","))
                    e = spec[stream]
                    if e == "alt":  # alternate HWDGE rings by parity
                        e = "sync" if t % 2 == 0 else "scalar"
                    return engines[e]
                if stream in ("x", "h", "l"):
                    if load_engine == "split":
                        return engines["scalar" if stream == "x" else "sync"]
                    return engines[load_engine]
                if store_engine == "mix":
                    return engines["scalar" if stream == "xo" else "gpsimd"]
                if store_engine == "alt":
                    return engines["gpsimd" if t % 2 == 0 else "scalar"]
                return engines[store_engine]

            # (offset, width) chunks; the last tile optionally split so the
            # final load->DVE->store chain is short (pipelines the drain).
            chunks = [(i * tile_cols, tile_cols) for i in range(n_tiles - 1)]
            if tail_split > 1:
                sub = tile_cols // tail_split
                base = (n_tiles - 1) * tile_cols
                chunks += [(base + j * sub, sub) for j in range(tail_split)]
            else:
                chunks.append(((n_tiles - 1) * tile_cols, tile_cols))

            def body():
                for t in range(len(chunks) * repeats):
                    off, width = chunks[t % len(chunks)]
                    sl = slice(off, off + width)
                    x_store = eng_for("xo", t)
                    store_ho = eng_for("ho", t)
                    store_lo = eng_for("lo", t)

                    xt = pool.tile([P, width], F32, tag="x")
                    eng_for("x", t).dma_start(out=xt[:], in_=x[:, sl])
                    if x_relu_on_dve:
                        nc.vector.tensor_scalar_max(xt[:], xt[:], 0.0)
                    else:
                        nc.scalar.activation(xt[:], xt[:], relu)
                    x_store.dma_start(out=x_out[:, sl], in_=xt[:])

                    ht = pool.tile([P, width], F32, tag="h")
                    eng_for("h", t).dma_start(out=ht[:], in_=high[:, sl])
                    lt = pool.tile([P, width], F32, tag="l")
                    eng_for("l", t).dma_start(out=lt[:], in_=low[:, sl])

                    nc.scalar.activation(ht[:], ht[:], relu)
                    store_ho.dma_start(out=high_out[:, sl], in_=ht[:])

                    tt = ht if inplace_low else pool.tile([P, width], F32, tag="t")
                    nc.vector.tensor_add(tt[:], lt[:], ht[:])
                    nc.vector.tensor_scalar(
                        tt[:], tt[:], 0.0, None, mybir.AluOpType.is_ge
                    )
                    nc.vector.tensor_mul(tt[:], tt[:], lt[:])
                    store_lo.dma_start(out=low_out[:, sl], in_=tt[:])

            if hw_loop_repeats > 1:
                with tc.For_i(0, hw_loop_repeats, 1):
                    body()
            else:
                body()
    nc.compile()
    return nc


_NC = None


def _get_nc() -> bass.Bass:
    global _NC
    if _NC is None:
        _NC = build_program()
    return _NC


def kernel(x: np.ndarray, low: np.ndarray, high: np.ndarray, **_run_kwargs):
    nc = _get_nc()
    x = np.ascontiguousarray(np.asarray(x, dtype=np.float32).reshape(-1))
    low = np.ascontiguousarray(np.asarray(low, dtype=np.float32).reshape(-1))
    high = np.ascontiguousarray(np.asarray(high, dtype=np.float32).reshape(-1))
    assert x.shape == (N,), x.shape
    in_maps = []
    for c in range(N_CORES):
        s = slice(c * SHARD, (c + 1) * SHARD)
        in_maps.append(
            {
                "x": x[s].reshape(P, FREE),
                "low": low[s].reshape(P, FREE),
                "high": high[s].reshape(P, FREE),
            }
        )
    res = None
    for attempt in range(3):
        try:
            res = run_bass_kernel_spmd(nc, in_maps, list(range(N_CORES)), **_run_kwargs)
            break
        except Exception:
            # Transient device wedge (NRT_EXEC_UNIT_UNRECOVERABLE) — reset the
            # jax backend so the next attempt re-establishes the device mesh.
            if attempt == 2:
                raise
            import time as _time

            try:
                import jax

                jax.clear_caches()
                jax.extend.backend.clear_backends()
            except Exception:
                pass
            _time.sleep(10.0)
    results = res.results
    x_out = np.concatenate([results[c]["x_out"].reshape(-1) for c in range(N_CORES)])
    low_out = np.concatenate([results[c]["low_out"].reshape(-1) for c in range(N_CORES)])
    high_out = np.concatenate([results[c]["high_out"].reshape(-1) for c in range(N_CORES)])
    if _run_kwargs:
        kernel.last_results = res  # expose trace/profile to test harness
    return (
        x_out.astype(np.float32, copy=False),
        low_out.astype(np.float32, copy=False),
        high_out.astype(np.float32, copy=False),
    )


# revision 27
# speedup vs baseline: 1.7273x; 1.1287x over previous
"""Trainium2 Bass kernel for nn_AbstractRelu (DeepPoly abstract-ReLU transform).

The reference's piecewise-linear transform reduces exactly to:
    x_out    = relu(x)
    high_out = relu(high)        (crossing branch: w_high*high + b_high == high)
    low_out  = low if low + high >= 0 else 0
and `relu(high)` can replace `high` in the low_out test without changing any
result (when high <= 0, low < high <= 0 forces low + high < 0 AND low < 0).

Sharding: N=16.7M elements split evenly across 8 NeuronCores; fully
elementwise, no communication.
"""

import numpy as np

import concourse.bass as bass
import concourse.bacc as bacc
import concourse.mybir as mybir
from concourse.tile import TileContext
from concourse.bass_utils import run_bass_kernel_spmd

N = 16777216
N_CORES = 8
SHARD = N // N_CORES  # 2_097_152
P = 128
FREE = SHARD // P  # 16384 f32 per partition per core (64 KiB)
TILE_COLS = 4096  # 2 MiB per DMA transfer
F32 = mybir.dt.float32


def build_program(
    free: int = FREE,
    tile_cols: int = TILE_COLS,
    bufs: int = 3,
    repeats: int = 1,
    hw_loop_repeats: int = 1,
    inplace_low: bool = False,
    store_engine: str = "gpsimd",
    load_engine: str = "split",
    dma_map: str | None = None,
    x_relu_on_dve: bool = False,
    tail_split: int = 1,
) -> bass.Bass:
    """hw_loop_repeats wraps the whole body in a tc.For_i hardware loop —
    used only by the timing harness (repeat-differencing).
    inplace_low computes low_out inside the high tile (3 SBUF tags instead
    of 4, allowing larger tiles)."""
    assert free % tile_cols == 0
    n_tiles = free // tile_cols

    nc = bacc.Bacc(
        "TRN2", target_bir_lowering=False, debug=False, num_devices=N_CORES
    )
    x = nc.declare_dram_parameter("x", [P, free], F32, isOutput=False)
    low = nc.declare_dram_parameter("low", [P, free], F32, isOutput=False)
    high = nc.declare_dram_parameter("high", [P, free], F32, isOutput=False)
    x_out = nc.declare_dram_parameter("x_out", [P, free], F32, isOutput=True)
    low_out = nc.declare_dram_parameter("low_out", [P, free], F32, isOutput=True)
    high_out = nc.declare_dram_parameter("high_out", [P, free], F32, isOutput=True)

    relu = mybir.ActivationFunctionType.Relu
    with TileContext(nc) as tc:
        with tc.tile_pool(name="io", bufs=bufs) as pool:
            engines = {"scalar": nc.scalar, "gpsimd": nc.gpsimd, "sync": nc.sync}

            def eng_for(stream: str, t: int):
                """Resolve the DMA-issuing engine for stream in
                {x,h,l,xo,ho,lo} at tile t."""
                if dma_map is not None:
                    spec = dict(kv.split(":") for kv in dma_map.split(","))
                    e = spec[stream]
                    if e == "alt":  # alternate HWDGE rings by parity
                        e = "sync" if t % 2 == 0 else "scalar"
                    return engines[e]
                if stream in ("x", "h", "l"):
                    if load_engine == "split":
                        return engines["scalar" if stream == "x" else "sync"]
                    return engines[load_engine]
                if store_engine == "mix":
                    return engines["scalar" if stream == "xo" else "gpsimd"]
                if store_engine == "alt":
                    return engines["gpsimd" if t % 2 == 0 else "scalar"]
                return engines[store_engine]

            # (offset, width) chunks; the last tile optionally split so the
            # final load->DVE->store chain is short (pipelines the drain).
            chunks = [(i * tile_cols, tile_cols) for i in range(n_tiles - 1)]
            if tail_split > 1:
                sub = tile_cols // tail_split
                base = (n_tiles - 1) * tile_cols
                chunks += [(base + j * sub, sub) for j in range(tail_split)]
            else:
                chunks.append(((n_tiles - 1) * tile_cols, tile_cols))

            def body():
                for t in range(len(chunks) * repeats):
                    off, width = chunks[t % len(chunks)]
                    sl = slice(off, off + width)
                    x_store = eng_for("xo", t)
                    store_ho = eng_for("ho", t)
                    store_lo = eng_for("lo", t)

                    xt = pool.tile([P, width], F32, tag="x")
                    eng_for("x", t).dma_start(out=xt[:], in_=x[:, sl])
                    if x_relu_on_dve:
                        nc.vector.tensor_scalar_max(xt[:], xt[:], 0.0)
                    else:
                        nc.scalar.activation(xt[:], xt[:], relu)
                    x_store.dma_start(out=x_out[:, sl], in_=xt[:])

                    ht = pool.tile([P, width], F32, tag="h")
                    eng_for("h", t).dma_start(out=ht[:], in_=high[:, sl])
                    lt = pool.tile([P, width], F32, tag="l")
                    eng_for("l", t).dma_start(out=lt[:], in_=low[:, sl])

                    nc.scalar.activation(ht[:], ht[:], relu)
                    store_ho.dma_start(out=high_out[:, sl], in_=ht[:])

                    tt = ht if inplace_low else pool.tile([P, width], F32, tag="t")
                    nc.vector.tensor_add(tt[:], lt[:], ht[:])
                    nc.vector.tensor_scalar(
                        tt[:], tt[:], 0.0, None, mybir.AluOpType.is_ge
                    )
                    nc.vector.tensor_mul(tt[:], tt[:], lt[:])
                    store_lo.dma_start(out=low_out[:, sl], in_=tt[:])

            if hw_loop_repeats > 1:
                with tc.For_i(0, hw_loop_repeats, 1):
                    body()
            else:
                body()
    nc.compile()
    return nc


_NC = None


def _get_nc() -> bass.Bass:
    global _NC
    if _NC is None:
        _NC = build_program()
    return _NC


_RUNNER = None


def _make_runner(nc):
    """Cached PJRT runner (mirrors bass2jax.run_bass_via_pjrt, but the jitted
    callable is built once so repeat kernel() calls skip re-tracing). No
    donation: this kernel writes every output element, so the zero 'output'
    operands are reusable dummies and XLA result buffers may start uninit."""
    import jax
    from jax.sharding import Mesh, PartitionSpec, NamedSharding
    from jax.experimental.shard_map import shard_map
    from concourse.bass2jax import (
        _bass_exec_p,
        install_neuronx_cc_hook,
        partition_id_tensor,
    )

    install_neuronx_cc_hook()
    partition_name = nc.partition_id_tensor.name if nc.partition_id_tensor else None

    in_names, out_names, out_avals, zero_shapes = [], [], [], []
    for alloc in nc.m.functions[0].allocations:
        if not isinstance(alloc, mybir.MemoryLocationSet):
            continue
        name = alloc.memorylocations[0].name
        if alloc.kind == "ExternalInput":
            if name != partition_name:
                in_names.append(name)
        elif alloc.kind == "ExternalOutput":
            shape = tuple(alloc.tensor_shape)
            dtype = mybir.dt.np(alloc.dtype)
            out_names.append(name)
            out_avals.append(jax.core.ShapedArray(shape, dtype))
            zero_shapes.append((shape, dtype))
    n_params = len(in_names)
    all_in_names = list(in_names) + list(out_names)
    if partition_name is not None:
        all_in_names.append(partition_name)

    def _body(*args):
        operands = list(args)
        if partition_name is not None:
            operands.append(partition_id_tensor())
        outs = _bass_exec_p.bind(
            *operands,
            out_avals=tuple(out_avals),
            in_names=tuple(all_in_names),
            out_names=tuple(out_names),
            lowering_input_output_aliases=(),
            sim_require_finite=True,
            sim_require_nnan=True,
            nc=nc,
        )
        return tuple(outs)

    devices = jax.devices()[:N_CORES]
    mesh = Mesh(np.asarray(devices), ("core",))
    n_io = n_params + len(out_names)
    sharded = jax.jit(
        shard_map(
            _body,
            mesh=mesh,
            in_specs=(PartitionSpec("core"),) * n_io,
            out_specs=(PartitionSpec("core"),) * len(out_names),
            check_rep=False,
        ),
        keep_unused=True,
    )
    sharding = NamedSharding(mesh, PartitionSpec("core"))
    zeros = [
        jax.device_put(np.zeros((N_CORES * s[0], *s[1:]), d), sharding)
        for (s, d) in zero_shapes
    ]

    def run(in_maps):
        concat_in = [
            np.concatenate([np.asarray(in_maps[c][nm]) for c in range(N_CORES)], axis=0)
            for nm in in_names
        ]
        dev_in = [jax.device_put(a, sharding) for a in concat_in]
        outs = sharded(*dev_in, *zeros)
        return {
            nm: np.asarray(outs[i]).reshape(N_CORES, *out_avals[i].shape)
            for i, nm in enumerate(out_names)
        }

    return run


def kernel(x: np.ndarray, low: np.ndarray, high: np.ndarray, **_run_kwargs):
    nc = _get_nc()
    x = np.ascontiguousarray(np.asarray(x, dtype=np.float32).reshape(-1))
    low = np.ascontiguousarray(np.asarray(low, dtype=np.float32).reshape(-1))
    high = np.ascontiguousarray(np.asarray(high, dtype=np.float32).reshape(-1))
    assert x.shape == (N,), x.shape
    in_maps = []
    for c in range(N_CORES):
        s = slice(c * SHARD, (c + 1) * SHARD)
        in_maps.append(
            {
                "x": x[s].reshape(P, FREE),
                "low": low[s].reshape(P, FREE),
                "high": high[s].reshape(P, FREE),
            }
        )
    global _RUNNER
    results = None
    if not _run_kwargs:
        # Fast path: cached jitted executable (no per-call re-trace).
        try:
            if _RUNNER is None:
                _RUNNER = _make_runner(nc)
            by_name = _RUNNER(in_maps)
            results = [
                {nm: by_name[nm][c] for nm in by_name} for c in range(N_CORES)
            ]
        except Exception:
            _RUNNER = None
            results = None

    if results is None:
        res = None
        for attempt in range(3):
            try:
                res = run_bass_kernel_spmd(
                    nc, in_maps, list(range(N_CORES)), **_run_kwargs
                )
                break
            except Exception:
                # Transient device wedge (NRT_EXEC_UNIT_UNRECOVERABLE) — reset
                # the jax backend so the next attempt re-establishes the mesh.
                if attempt == 2:
                    raise
                import time as _time

                try:
                    import jax

                    jax.clear_caches()
                    jax.extend.backend.clear_backends()
                except Exception:
                    pass
                _time.sleep(10.0)
        results = res.results
        if _run_kwargs:
            kernel.last_results = res  # expose trace/profile to test harness

    x_out = np.concatenate([results[c]["x_out"].reshape(-1) for c in range(N_CORES)])
    low_out = np.concatenate([results[c]["low_out"].reshape(-1) for c in range(N_CORES)])
    high_out = np.concatenate([results[c]["high_out"].reshape(-1) for c in range(N_CORES)])
    return (
        x_out.astype(np.float32, copy=False),
        low_out.astype(np.float32, copy=False),
        high_out.astype(np.float32, copy=False),
    )
